# revision 1
# baseline (speedup 1.0000x reference)
"""Contrastive loss (cosine-sim InfoNCE with positive pairs) on 8 TRN2 NeuronCores.

Math: per row i, with sim = cos-sim matrix and tau = 0.08,
  loss = mean_i [ log( sum_j exp(sim_ij/tau) - exp(sim_ii/tau) ) - sim_{i,p(i)}/tau ]
where p(i) is i's positive partner. (The masked denominator pos+row_sums
telescopes to total - diag.)

Sharding: data-parallel over rows. Each core gets the full embeddings (for the
rhs of the Gram matmul) plus its 1024-row slice and the partner-gathered slice
(host-side index plumbing only). Each core computes its [1024 x 8192] slice of
exp(sim/tau) row sums streaming through PSUM (never materializing the matrix),
plus its per-row diag/pos corrections and log terms, and writes a [128,1]
vector of partial loss sums. Host sums 8*128 partials and divides by B.

ACT-engine discipline (the bottleneck): every 1/||e|| is computed on the DVE
with a Newton rsqrt (fixed seed 128^-0.5 is accurate because ||e||^2 ~
chi2_128 is concentrated), so the Activation engine runs ONE table load, a
pure exp stream (exp in place over PSUM + hardware row-sum accumulator), and a
single trailing Ln - no Ln/Exp table thrash.

DMA discipline: the HWDGE/DMA device serializes instructions (~625ns fixed +
transfer each), so transposes are batched 8 row-tiles per dma_start_transpose
(the xbar transposes [128, n*128] -> n tile-transposes in one instruction) and
the cold-start order is [batch0 | local | partner | batch1], with later
batches held back via tile_wait_until so they can't crowd the critical path.

Numerics: the Gram matmul runs in fp16 (rhs = normalized embeddings, lhsT = raw
rows; the exp's per-partition scale applies rinv_i/tau). The diagonal exp must
cancel against the same value inside the accumulated row total, so it is
recomputed from the *same* fp16 tensors with a DVE dot product, and rinv for
the local rows is produced by the bit-identical DVE op sequence used for the
full-matrix rinv (norms live in one [local | full | partner] buffer so the
joint Newton runs on one contiguous slice).
"""

import numpy as np

import concourse.bacc as bacc
import concourse.bass_utils as bass_utils
import concourse.mybir as mybir
import concourse.tile as tile
from concourse.dve_ops import AFFINE_MUL_REDUCE

B, D = 8192, 128
N_CORES = 8
ROWS = B // N_CORES            # 1024 rows per core
P = 128                        # partitions
T_FULL = B // P                # 64 row-tiles of the full matrix
T_LOC = ROWS // P              # 8 row-tiles per core
N_CHUNK = 512                  # matmul free dim (one PSUM bank)
TAU = 0.08

# column groups of the main loop: first two are single-batch (1024 cols) so
# the exp stream starts as soon as one 8-tile preproc batch is done; the rest
# are 2048-wide to amortize ACT per-instruction overhead.
GROUPS = [(0, 8), (8, 16), (16, 32), (32, 48), (48, 64)]   # (tile0, tile1)
N_GRPS = len(GROUPS)

# norm-buffer column layout: [ local 0:8 | full tiles 8:72 | partner 72:80 ]
NL, NF, NP = 0, T_LOC, T_LOC + T_FULL

# Newton rsqrt seed: y0 = 128^-0.5 (rows are ~N(0,1)^128 so ss ~ 128 +- 20%)
_SEED = float(128.0 ** -0.5)
_AFF_A = -0.5 * _SEED ** 3     # iter-1 collapses to an affine: y1 = A*ss + B
_AFF_B = 1.5 * _SEED

f32 = mybir.dt.float32
f16 = mybir.dt.float16
AF = mybir.ActivationFunctionType
ALU = mybir.AluOpType
AX = mybir.AxisListType

_cache = {}


def _build():
    nc = bacc.Bacc("TRN2", target_bir_lowering=False, debug=False,
                   num_devices=N_CORES)
    ef = nc.dram_tensor("e_full", [B, D], f32, kind="ExternalInput").ap()
    el = nc.dram_tensor("e_loc", [ROWS, D], f32, kind="ExternalInput").ap()
    ep = nc.dram_tensor("e_par", [ROWS, D], f32, kind="ExternalInput").ap()
    out = nc.dram_tensor("partial", [P, 1], f32, kind="ExternalOutput").ap()
    # DRAM bounce buffers for the batched xbar transposes: SBUF->SBUF
    # multi-tile dma_start_transpose reads the wrong tile (validated on HW),
    # while DRAM->SBUF full-matrix transposes are the production path.
    scr_en = nc.dram_tensor("scr_en", [B, D], f16, kind="Internal").ap()
    scr_lhs = nc.dram_tensor("scr_lhs", [ROWS, D], f16, kind="Internal").ap()

    with tile.TileContext(nc) as tc:
        with (
            tc.tile_pool(name="big", bufs=1) as big,
            tc.tile_pool(name="sq", bufs=2) as sqp,
            tc.tile_pool(name="small", bufs=1) as sm,
            tc.tile_pool(name="psum", bufs=2, space="PSUM") as pp,
        ):
            # ---- persistent SBUF tensors ----
            ef32 = big.tile([P, T_FULL, D], f32)       # full E, natural tiles
            ent = big.tile([P, B], f16)                # EN^T  (d-part, row-free)
            eloc32 = sm.tile([P, T_LOC, D], f32)
            epar32 = sm.tile([P, T_LOC, D], f32)
            eloc16 = sm.tile([P, T_LOC, D], f16)       # raw local rows, fp16
            enloc16 = sm.tile([P, T_LOC, D], f16)      # normalized local rows
            lhsT = sm.tile([P, ROWS], f16)             # (raw local rows)^T
            nrm = sm.tile([P, 80], f32)                # ||e||^2 [loc|full|par]
            rin = sm.tile([P, 80], f32)                # 1/||e||  same layout
            rinv_ls = sm.tile([P, T_LOC], f32)         # 1/(tau*||e||) (local)
            diag = sm.tile([P, T_LOC], f32)            # raw diag dots (fp16 in)
            d2 = sm.tile([P, T_LOC], f32)
            posdot = sm.tile([P, T_LOC], f32)          # raw pos dots (fp32)
            posfac = sm.tile([P, T_LOC], f32)
            pos2 = sm.tile([P, T_LOC], f32)
            dexp = sm.tile([P, T_LOC], f32)
            acc = sm.tile([P, T_LOC * N_GRPS], f32)    # exp row-sums per group
            rtot = sm.tile([P, T_LOC], f32)
            denom = sm.tile([P, T_LOC], f32)
            lvec = sm.tile([P, T_LOC], f32)
            lossv = sm.tile([P, T_LOC], f32)
            part = sm.tile([P, 1], f32)

            rinv_loc = rin[:, NL:NL + T_LOC]
            rinv_par = rin[:, NP:NP + T_LOC]

            def newton_rsqrt(c0, c1):
                """rin[:, c0:c1] = 1/sqrt(nrm[:, c0:c1]) on DVE. One affine +
                3 Newton steps; elementwise fp32, so equal inputs give
                bit-equal outputs regardless of which slice they sit in."""
                n = c1 - c0
                dst = rin[:, c0:c1]
                src = nrm[:, c0:c1]
                ya = sqp.tile([P, n], f32, tag=f"nw{n}a")
                yb = sqp.tile([P, n], f32, tag=f"nw{n}b")
                yt = sqp.tile([P, n], f32, tag=f"nw{n}t")
                nc.vector.tensor_scalar(yt[:], src, _AFF_A, _AFF_B,
                                        op0=ALU.mult, op1=ALU.add)
                cur = yt[:]
                for it in range(3):
                    nxt = yt[:] if it % 2 else dst
                    nc.vector.tensor_mul(ya, cur, cur)
                    nc.vector.tensor_mul(yb, src, ya)
                    nc.vector._custom_dve(AFFINE_MUL_REDUCE, out=nxt, in0=yb,
                                          in1=cur, s0=-0.5, s1=1.5)
                    cur = nxt

            def norms(dst_c0, src32, t0, t1):
                """nrm[:, dst_c0:dst_c0+(t1-t0)] = row norms^2 of src tiles."""
                n = t1 - t0
                sq = sqp.tile([P, n, D], f32, tag=f"sq{n}")
                nc.vector.tensor_mul(sq[:], src32[:, t0:t1, :],
                                     src32[:, t0:t1, :])
                nc.vector.reduce_sum(nrm[:, dst_c0:dst_c0 + n], sq[:],
                                     axis=AX.X)

            enb_tiles = {}

            def scale_batch(t0, t1):
                """enb tiles [t0,t1) = normalized fp16 rows (pre-transpose)."""
                n = t1 - t0
                enb = sqp.tile([P, n, D], f16, tag=f"en{t0}", bufs=1)
                enb_tiles[t0] = enb
                for t in range(t0, t1):
                    nc.vector.tensor_scalar_mul(enb[:, t - t0, :],
                                                ef32[:, t, :],
                                                rin[:, NF + t:NF + t + 1])

            scr_en_r = scr_en.rearrange("(t p) d -> p t d", p=P)

            def transpose_batch(t0, t1, eng=None):
                e = eng or nc.sync
                e.dma_start(out=scr_en_r[:, t0:t1, :], in_=enb_tiles[t0][:])
                e.dma_start_transpose(ent[:, t0 * P:t1 * P],
                                      scr_en[t0 * P:t1 * P, :])

            def main_phase(gi):
                t0, t1 = GROUPS[gi]
                w = (t1 - t0) * P
                for m in range(T_LOC):
                    lhs_m = lhsT[:, m * P:(m + 1) * P]
                    pt = pp.tile([P, 2048], f32, tag="pt")
                    for k in range(w // N_CHUNK):
                        c0 = t0 * P + k * N_CHUNK
                        nc.tensor.matmul(
                            pt[:, k * N_CHUNK:(k + 1) * N_CHUNK],
                            lhsT=lhs_m,
                            rhs=ent[:, c0:c0 + N_CHUNK],
                            start=True, stop=True)
                    # exp in place in PSUM; row-sum via the ACT accumulator
                    nc.scalar.activation(
                        pt[:, :w], pt[:, :w], AF.Exp,
                        scale=rinv_ls[:, m:m + 1],
                        accum_out=acc[:, m * N_GRPS + gi:m * N_GRPS + gi + 1])

            # ---- cold start: batch 0 first, in critical-path order -------
            el_r = el.rearrange("(t p) d -> p t d", p=P)
            ep_r = ep.rearrange("(t p) d -> p t d", p=P)
            ef_r = ef.rearrange("(t p) d -> p t d", p=P)

            def held_dma(ms, dst, src):
                with tc.tile_wait_until(ms):
                    nc.sync.dma_start(out=dst, in_=src)

            # SP queue order (in-order, parks on waits -> batch DMAs, which
            # never wait, go first; transposes follow in completion order)
            nc.sync.dma_start(out=ef32[:, 0:8, :], in_=ef_r[:, 0:8, :])
            nc.sync.dma_start(out=eloc32[:], in_=el_r)
            held_dma(0.009, ef32[:, 8:16, :], ef_r[:, 8:16, :])
            held_dma(0.0125, ef32[:, 16:32, :], ef_r[:, 16:32, :])
            held_dma(0.016, epar32[:], ep_r)

            # fp32 -> fp16 cast on the (idle) scalar engine: Copy is in every
            # activation table, so this costs no extra table load and keeps
            # the DVE critical chain (norms -> newton -> scales) unbroken.
            nc.scalar.copy(eloc16[:], eloc32[:])
            nc.scalar.dma_start(
                out=scr_lhs.rearrange("(t p) d -> p t d", p=P),
                in_=eloc16[:])
            nc.scalar.dma_start_transpose(lhsT[:], scr_lhs)

            # head-critical chain: batch-0 norms -> newton -> scales -> xbar
            norms(NF, ef32, 0, 8)
            newton_rsqrt(NF, NF + 8)
            scale_batch(0, 8)
            transpose_batch(0, 8, eng=nc.scalar)

            # local norms: gate only the exp scale, which is needed ~3.5us
            # after the batch-0 scales (transpose+matmul sit in between)
            with tc.tile_wait_until(0.0105):
                norms(NL, eloc32, 0, T_LOC)
                newton_rsqrt(NL, NL + T_LOC)
                nc.vector.tensor_scalar_mul(rinv_ls[:], rinv_loc, 1.0 / TAU)
            # normalized local rows (same op/engine as ent scaling: the fp16
            # values must match the matmul rhs bit-for-bit)
            with tc.tile_wait_until(0.016):
                for m in range(T_LOC):
                    nc.vector.tensor_scalar_mul(enloc16[:, m, :],
                                                eloc32[:, m, :],
                                                rinv_loc[:, m:m + 1])

            with tc.tile_wait_until(0.010):
                norms(NF + 8, ef32, 8, 16)
                newton_rsqrt(NF + 8, NF + 16)
                scale_batch(8, 16)
            transpose_batch(8, 16)

            held_dma(0.019, ef32[:, 32:48, :], ef_r[:, 32:48, :])
            held_dma(0.023, ef32[:, 48:64, :], ef_r[:, 48:64, :])

            main_phase(0)

            with tc.tile_wait_until(0.019):
                norms(NF + 16, ef32, 16, 32)
                newton_rsqrt(NF + 16, NF + 32)
                scale_batch(16, 24)
                scale_batch(24, 32)
            transpose_batch(16, 24)
            transpose_batch(24, 32)

            main_phase(1)

            with tc.tile_wait_until(0.028):
                norms(NF + 32, ef32, 32, 48)
                newton_rsqrt(NF + 32, NF + 48)
                scale_batch(32, 40)
                scale_batch(40, 48)
            transpose_batch(32, 40)
            transpose_batch(40, 48)

            # partner norms + pos/diag terms: DVE slack mid-stream, and the
            # dexp exp rides the main exp stream (same ACT table).
            with tc.tile_wait_until(0.024):
                norms(NP, epar32, 0, T_LOC)
                newton_rsqrt(NP, NP + T_LOC)
                dprod = sqp.tile([P, T_LOC, D], f32, tag="sq8")
                nc.vector.tensor_mul(dprod[:], eloc16[:], enloc16[:])
                nc.vector.reduce_sum(diag[:], dprod[:], axis=AX.X)
                nc.vector.tensor_mul(d2[:], diag[:], rinv_ls[:])
                nc.scalar.activation(dexp[:], d2[:], AF.Exp)
                pprod = sqp.tile([P, T_LOC, D], f32, tag="sq8")
                nc.vector.tensor_mul(pprod[:], eloc32[:], epar32[:])
                nc.vector.reduce_sum(posdot[:], pprod[:], axis=AX.X)
                nc.vector.tensor_mul(posfac[:], rinv_ls[:], rinv_par)
                nc.vector.tensor_mul(pos2[:], posdot[:], posfac[:])

            main_phase(2)

            with tc.tile_wait_until(0.036):
                norms(NF + 48, ef32, 48, 64)
                newton_rsqrt(NF + 48, NF + 64)
                scale_batch(48, 56)
                scale_batch(56, 64)
            transpose_batch(48, 56)
            transpose_batch(56, 64)

            main_phase(3)
            main_phase(4)

            # ---- epilogue: per-row loss, reduce to [128,1] ---------------
            acc_v = acc[:].rearrange("p (m g) -> p m g", g=N_GRPS)
            nc.vector.reduce_sum(rtot[:], acc_v, axis=AX.X)
            nc.vector.tensor_tensor(out=denom[:], in0=rtot[:], in1=dexp[:],
                                    op=ALU.subtract)
            nc.scalar.activation(lvec[:], denom[:], AF.Ln)
            nc.vector.tensor_tensor(out=lossv[:], in0=lvec[:], in1=pos2[:],
                                    op=ALU.subtract)
            nc.vector.reduce_sum(part[:], lossv[:], axis=AX.X)
            nc.sync.dma_start(out=out, in_=part[:])

    nc.compile()
    return nc


def _get_nc():
    if "nc" not in _cache:
        _cache["nc"] = _build()
    return _cache["nc"]


def kernel(embeddings, positive_pairs):
    E = np.ascontiguousarray(np.asarray(embeddings), dtype=np.float32)
    pp = np.asarray(positive_pairs)
    assert E.shape == (B, D)

    partner = np.full(B, -1, dtype=np.int64)
    i, j = pp[:, 0].astype(np.int64), pp[:, 1].astype(np.int64)
    partner[i] = j
    partner[j] = i
    assert (partner >= 0).all(), "positive_pairs must cover every row"

    nc = _get_nc()
    in_maps = []
    for c in range(N_CORES):
        rows = np.arange(c * ROWS, (c + 1) * ROWS)
        in_maps.append({
            "e_full": E,
            "e_loc": E[rows],
            "e_par": np.ascontiguousarray(E[partner[rows]]),
        })
    res = bass_utils.run_bass_kernel_spmd(nc, in_maps,
                                          core_ids=list(range(N_CORES)))
    total = sum(float(res.results[c]["partial"].sum()) for c in range(N_CORES))
    return np.float32(total / B)



# revision 2
# speedup vs baseline: 1.1362x; 1.1362x over previous
"""Contrastive loss (cosine InfoNCE) on 8 TRN2 cores — v2.

Per-core: load full E fp32 + local/partner slices; fused square+reduce norms
(custom DVE TENSOR_TENSOR_REDUCE per tile), Newton rsqrt; normalize+cast to
fp8e4 scaled by sqrt(1/(4*tau)) (Pool + ACT bootstrap); PE tile-transposes
into the PSUM stream slots (no DRAM bounce), DVE copies fp8 (as fp16 views,
2x mode) to SBUF; fp8 matmuls stream [128,2048] chunks; exp+row-sum split
between ACT (Exp, scale=4, fused accum) and a custom DVE op EXP4
(lam*[(c0 z + c1)(z + c2)^2]^4 ~ exp(4z), fused accum) by row-tile so the
diagonal correction can be recomputed with the matching flavor.
denom_i = rowsum_i - diag_i (positives cancel); loss = log(denom) - pos/tau.
"""

import math
from operator import add

import numpy as np

import concourse.bacc as bacc
import concourse.bass_utils as bass_utils
import concourse.mybir as mybir
import concourse.tile as tile
import concourse.dve_ops as dve_ops
from concourse.dve_ops import TENSOR_TENSOR_REDUCE
from concourse.dve_spec import Spec, Src0, C0, C1, C2, lower
from concourse.dve_uop import DveOpSpec

B, D = 8192, 128
N_CORES = 8
ROWS = B // N_CORES
P = 128
T_FULL = 64
T_LOC = 8
TAU = 0.08
GW = 2048                 # stream column-group width
N_G = B // GW             # 4 groups
TPG = GW // P             # 16 tiles per group

# exp(4z) ~ exp(LOGLAM) * [(C0F z + C1F)(z + C2F)^2]^4, z = sim/(4 tau)
C0F, C1F, C2F = 0.6341681, 1.74306382, 2.74858327
LOGLAM = -10.304842156137992
SQK = math.sqrt(1.0 / (4.0 * TAU))   # fp8 cast scale per side
N_DVE = 3                 # row-tiles 5,6,7 exp'd on DVE; 0-4 on ACT

f32 = mybir.dt.float32
f16 = mybir.dt.float16
f8 = mybir.dt.float8e4  # unused
AF = mybir.ActivationFunctionType
ALU = mybir.AluOpType
AX = mybir.AxisListType

_cache = {}


def _ref_exp4(in0, in1, c0, c1, c2):
    x = in0.astype(np.float32)
    a = ((np.float32(c0) * x).astype(np.float32) + np.float32(c1)).astype(np.float32)
    b = (x + np.float32(c2)).astype(np.float32)
    u = (a * (b * b).astype(np.float32)).astype(np.float32)
    u = (u * u).astype(np.float32)
    u = (u * u).astype(np.float32)
    cs = np.cumsum(u.reshape(u.shape[0], -1), axis=-1, dtype=np.float32)
    return u, cs[:, -1:]


def _make_exp4_op():
    if "EXP4_CUBE_REDUCE" in dve_ops._SUB_OPCODE_FOR_NAME:
        for op in dve_ops.OPS:
            if op.name == "EXP4_CUBE_REDUCE":
                return op
    s = Src0
    bnode = s + C2
    u = (C0 * s + C1) * (bnode * bnode)
    y = u * u
    y = y * y
    spec = Spec(body=y, accum=add, accum_init=None, reference=_ref_exp4)
    shas = {}
    for ver in ("v3", "v4"):
        uops = lower(spec, ver=ver)
        shas[ver] = DveOpSpec(name="EXP4_CUBE_REDUCE", opcode=0, uops=uops,
                              rd1_en=False).sha(ver)
    op = dve_ops.DveOp("EXP4_CUBE_REDUCE", spec, subdim=False, uops_sha=shas)
    dve_ops.OPS.append(op)
    dve_ops.CUSTOM_DVE_SPECS[op.name] = spec
    dve_ops._SUB_OPCODE_FOR_NAME[op.name] = max(
        dve_ops._SUB_OPCODE_FOR_NAME.values()) + 1
    return op


EXP4 = _make_exp4_op()

# norm-buffer layout: [ full 0:64 | local 64:72 | partner 72:80 ]
NF, NL, NP = 0, 64, 72


def _build():
    nc = bacc.Bacc("TRN2", target_bir_lowering=False, debug=False,
                   num_devices=N_CORES)
    ef = nc.dram_tensor("e_full", [B, D], f32, kind="ExternalInput").ap()
    el = nc.dram_tensor("e_loc", [ROWS, D], f32, kind="ExternalInput").ap()
    ep = nc.dram_tensor("e_par", [ROWS, D], f32, kind="ExternalInput").ap()
    ident = nc.dram_tensor("ident", [P, P], f32, kind="ExternalInput").ap()
    out = nc.dram_tensor("partial", [P, 1], f32, kind="ExternalOutput").ap()

    with tile.TileContext(nc) as tc:
        with (
            tc.tile_pool(name="big", bufs=1) as big,
            tc.tile_pool(name="sm", bufs=1) as sm,
            tc.tile_pool(name="ps", bufs=4, space="PSUM") as pp,
        ):
            ef32 = big.tile([P, T_FULL, D], f32)
            en16 = big.tile([P, T_FULL, D], f16)      # normalized*SQK, natural
            ent = big.tile([P, B], f16)              # transposed rhs
            el32 = sm.tile([P, T_LOC, D], f32)
            ep32 = sm.tile([P, T_LOC, D], f32)
            el16 = sm.tile([P, T_LOC, D], f16)
            lhsT = sm.tile([P, ROWS], f16)
            id16 = sm.tile([P, P], f16)
            idf = sm.tile([P, P], f32)
            nrm = sm.tile([P, 80], f32)
            rin = sm.tile([P, 80], f32)
            scl = sm.tile([P, 80], f32)             # rin * SQK
            escr = sm.tile([P, GW], f32)            # DVE exp out scratch
            acc_a = sm.tile([P, (T_LOC - N_DVE) * 8], f32)
            acc_d = sm.tile([P, N_DVE * 8], f32)
            diagz = sm.tile([P, T_LOC], f32)
            dex = sm.tile([P, T_LOC], f32)
            posd = sm.tile([P, T_LOC], f32)
            posf = sm.tile([P, T_LOC], f32)
            pos2 = sm.tile([P, T_LOC], f32)
            rtot = sm.tile([P, T_LOC], f32)
            rt_d = sm.tile([P, N_DVE], f32)
            denom = sm.tile([P, T_LOC], f32)
            lvec = sm.tile([P, T_LOC], f32)
            lossv = sm.tile([P, T_LOC], f32)
            part = sm.tile([P, 1], f32)

            ef_r = ef.rearrange("(t p) d -> p t d", p=P)
            el_r = el.rearrange("(t p) d -> p t d", p=P)
            ep_r = ep.rearrange("(t p) d -> p t d", p=P)

            # ---- loads ----
            nc.sync.dma_start(out=el32[:], in_=el_r)
            nc.sync.dma_start(out=ef32[:, 0:16, :], in_=ef_r[:, 0:16, :])
            nc.sync.dma_start(out=idf[:], in_=ident)
            nc.sync.dma_start(out=ep32[:], in_=ep_r)
            nc.sync.dma_start(out=ef32[:, 16:40, :], in_=ef_r[:, 16:40, :])
            nc.sync.dma_start(out=ef32[:, 40:64, :], in_=ef_r[:, 40:64, :])
            nc.vector.tensor_copy(id16[:], idf[:])

            def norms_ttr(dst_c, src, t0, t1):
                for t in range(t0, t1):
                    nc.vector._custom_dve(
                        TENSOR_TENSOR_REDUCE, out=escr[:, 0:D],
                        in0=src[:, t, :], in1=src[:, t, :], s0=0.0, s1=1.0,
                        accum_out=nrm[:, dst_c + t - t0:dst_c + t - t0 + 1])

            def newton(c0, c1):
                n = c1 - c0
                dst = rin[:, c0:c1]
                src = nrm[:, c0:c1]
                ya = sm.tile([P, n], f32, tag=f"nw{c0}a")
                yb = sm.tile([P, n], f32, tag=f"nw{c0}b")
                seed = float(D ** -0.5)
                nc.vector.tensor_scalar(dst, src, -0.5 * seed ** 3, 1.5 * seed,
                                        op0=ALU.mult, op1=ALU.add)
                for _ in range(3):
                    nc.vector.tensor_mul(ya, dst, dst)
                    nc.vector.tensor_mul(yb, src, ya)
                    nc.vector.tensor_scalar(yb, yb, -0.5, 1.5,
                                            op0=ALU.mult, op1=ALU.add)
                    nc.vector.tensor_mul(dst, dst, yb)
                nc.vector.tensor_scalar_mul(scl[:, c0:c1], dst, SQK)

            # local + first-16 norms first (head-critical)
            norms_ttr(NL, el32, 0, T_LOC)
            norms_ttr(NF, ef32, 0, 16)
            newton(NL, NL + T_LOC)
            newton(NF, NF + 16)

            # local cast on ACT (bootstrap), local transposes via PSUM
            for t in range(T_LOC):
                nc.scalar.activation(el16[:, t, :], el32[:, t, :], AF.Copy,
                                     scale=scl[:, NL + t:NL + t + 1])
            ltp = pp.tile([P, 1024], f32, tag="ch")
            ltp16 = ltp[:, 0:512].bitcast(f16)
            for t in range(T_LOC):
                nc.tensor.transpose(ltp16[:, t * P:(t + 1) * P],
                                    el16[:, t, :], id16[:])
            nc.vector.tensor_copy(lhsT[:], ltp16[:])

            # full casts: first 16 on ACT (bootstrap), rest on Pool
            def cast(t0, t1, eng):
                for t in range(t0, t1):
                    if eng == "act":
                        nc.scalar.activation(en16[:, t, :], ef32[:, t, :],
                                             AF.Copy,
                                             scale=scl[:, NF + t:NF + t + 1])
                    else:
                        nc.gpsimd.tensor_scalar_mul(en16[:, t, :], ef32[:, t, :],
                                                    scl[:, NF + t:NF + t + 1])

            cast(0, 16, "act")

            norms_ttr(NP, ep32, 0, T_LOC)
            norms_ttr(NF + 16, ef32, 16, 40)
            newton(NP, NP + T_LOC)
            newton(NF + 16, NF + 40)
            cast(16, 40, "pool")
            norms_ttr(NF + 40, ef32, 40, 64)
            newton(NF + 40, NF + 64)
            cast(40, 64, "pool")

            # diag-z: fused square-sum of local fp8 rows (fp32 accum),
            # value-identical to the stream diagonal
            for t in range(T_LOC):
                nc.vector._custom_dve(
                    TENSOR_TENSOR_REDUCE, out=escr[:, 0:D],
                    in0=el16[:, t, :], in1=el16[:, t, :], s0=0.0, s1=1.0,
                    accum_out=diagz[:, t:t + 1])
            # pos dots (exact fp32) and pos2 = <e,epar>*rinv*rinv_par/tau
            for t in range(T_LOC):
                nc.vector._custom_dve(
                    TENSOR_TENSOR_REDUCE, out=escr[:, 0:D],
                    in0=el32[:, t, :], in1=ep32[:, t, :], s0=0.0, s1=1.0,
                    accum_out=posd[:, t:t + 1])
            nc.vector.tensor_mul(posf[:], rin[:, NL:NL + T_LOC],
                                 rin[:, NP:NP + T_LOC])
            nc.vector.tensor_scalar_mul(posf[:], posf[:], 1.0 / TAU)
            nc.vector.tensor_mul(pos2[:], posd[:], posf[:])

            # dexp per flavor
            nc.scalar.activation(dex[:, 0:T_LOC - N_DVE],
                                 diagz[:, 0:T_LOC - N_DVE], AF.Exp, scale=4.0)
            nc.vector._custom_dve(
                EXP4, out=escr[:, 0:N_DVE], in0=diagz[:, T_LOC - N_DVE:T_LOC],
                s0=C0F, s1=C1F, imm2=C2F)
            nc.vector.tensor_copy(dex[:, T_LOC - N_DVE:T_LOC],
                                  escr[:, 0:N_DVE])

            # ---- main stream ----
            # 4 PSUM slots of [128,1024] f32; transpose units (8 tiles ->
            # 1024 f16 ent cols) claim a slot and use its first half.
            CHUNKS = [(i * 1024, 1024) for i in range(8)]
            NCH = len(CHUNKS)
            units_done = 0

            def emit_unit(u):
                tp = pp.tile([P, 1024], f32, tag="ch")
                tp16 = tp[:, 0:512].bitcast(f16)
                for t in range(8):
                    gt = u * 8 + t
                    nc.tensor.transpose(tp16[:, t * P:(t + 1) * P],
                                        en16[:, gt, :], id16[:])
                nc.vector.tensor_copy(ent[:, u * 1024:(u + 1) * 1024], tp16[:])

            for ci, (c0, cw) in enumerate(CHUNKS):
                need = (c0 + cw + 1023) // 1024
                while units_done < need:
                    emit_unit(units_done)
                    units_done += 1
                for m in (0, 5, 1, 6, 2, 7, 3, 4):
                    mt = pp.tile([P, 1024], f32, tag="ch")
                    for k in range(cw // 512):
                        nc.tensor.matmul(
                            mt[:, k * 512:(k + 1) * 512],
                            lhsT=lhsT[:, m * P:(m + 1) * P],
                            rhs=ent[:, c0 + k * 512:c0 + (k + 1) * 512],
                            start=True, stop=True)
                    if m < T_LOC - N_DVE:
                        nc.scalar.activation(
                            mt[:, 0:cw], mt[:, 0:cw], AF.Exp, scale=4.0,
                            accum_out=acc_a[:, m * NCH + ci:m * NCH + ci + 1])
                    else:
                        md = m - (T_LOC - N_DVE)
                        nc.vector._custom_dve(
                            EXP4, out=escr[:, 0:cw], in0=mt[:, 0:cw],
                            s0=C0F, s1=C1F, imm2=C2F,
                            accum_out=acc_d[:, md * NCH + ci:md * NCH + ci + 1])

            # ---- epilogue ----
            acc_av = acc_a[:].rearrange("p (m g) -> p m g", g=8)
            nc.vector.reduce_sum(rtot[:, 0:T_LOC - N_DVE], acc_av, axis=AX.X)
            acc_dv = acc_d[:].rearrange("p (m g) -> p m g", g=8)
            nc.vector.reduce_sum(rt_d[:], acc_dv, axis=AX.X)
            nc.vector.tensor_copy(rtot[:, T_LOC - N_DVE:T_LOC], rt_d[:])
            nc.vector.tensor_tensor(out=denom[:], in0=rtot[:], in1=dex[:],
                                    op=ALU.subtract)
            nc.scalar.activation(lvec[:], denom[:], AF.Ln)
            # DVE-flavor rows: log(denom_raw) + LOGLAM = log(denom_true)
            nc.vector.tensor_scalar(lvec[:, T_LOC - N_DVE:T_LOC],
                                    lvec[:, T_LOC - N_DVE:T_LOC], 1.0, LOGLAM,
                                    op0=ALU.mult, op1=ALU.add)
            nc.vector.tensor_tensor(out=lossv[:], in0=lvec[:], in1=pos2[:],
                                    op=ALU.subtract)
            nc.vector.reduce_sum(part[:], lossv[:], axis=AX.X)
            nc.sync.dma_start(out=out, in_=part[:])

    nc.compile()
    return nc


def _get_nc():
    if "nc" not in _cache:
        _cache["nc"] = _build()
    return _cache["nc"]


def kernel(embeddings, positive_pairs):
    E = np.ascontiguousarray(np.asarray(embeddings), dtype=np.float32)
    pp_arr = np.asarray(positive_pairs)
    assert E.shape == (B, D)

    partner = np.full(B, -1, dtype=np.int64)
    i, j = pp_arr[:, 0].astype(np.int64), pp_arr[:, 1].astype(np.int64)
    partner[i] = j
    partner[j] = i
    assert (partner >= 0).all()

    ident = np.eye(P, dtype=np.float32)
    nc = _get_nc()
    in_maps = []
    for c in range(N_CORES):
        rows = np.arange(c * ROWS, (c + 1) * ROWS)
        in_maps.append({
            "e_full": E,
            "e_loc": E[rows],
            "e_par": np.ascontiguousarray(E[partner[rows]]),
            "ident": ident,
        })
    res = bass_utils.run_bass_kernel_spmd(nc, in_maps,
                                          core_ids=list(range(N_CORES)))
    total = sum(float(res.results[c]["partial"].sum()) for c in range(N_CORES))
    return np.float32(total / B)
